# revision 11
# baseline (speedup 1.0000x reference)
"""Trainium2 Bass kernel for nn_ADLS_13022340842024 (moe_routing).

Data-parallel over batch across 8 NeuronCores (2048 samples/core).

Key algorithmic reductions (host-side, weight/index-only prep):
  * The gated domain-relation matrix Rg is a row-normalized diagonal =>
    h_prime = dom_emb[domain_id] exactly; all hierarchical routing (zeta,
    alpha) is therefore a function of domain_id only -> tiny [D,L]/[D,L,E]
    tables folded with SCALING into per-layer LoRA scale tables [D, E*R].
  * Per-domain towers flattened to one [512,64] matmul + block-diagonal
    [64,8] second layer + one-hot select.

On-device per core:
  * Embedding gather via dma_gather from per-half-batch compacted bf16
    tables (256B rows), PE col-packed transposes -> x_T [feat, batch].
  * 3-layer FCN backbone as feature-major matmuls (bf16 L0/L1, float32r
    L2) with LoRA experts folded into the PSUM accumulation, ACT
    relu+bias epilogues.
  * Towers + one-hot domain select, PE ones-reduce.
"""
import numpy as np
import ml_dtypes
from contextlib import ExitStack

import concourse.bass as bass
import concourse.tile as tile
from concourse import bacc, mybir
from concourse import bass_utils
from concourse.masks import make_identity

BF16 = ml_dtypes.bfloat16

B, F, V, ED = 16384, 32, 100000, 32
NCORES = 8
BL = B // NCORES                 # 2048 samples per core
IN, D0, D1, D2 = 1024, 2048, 1024, 512
D, E, L, R = 8, 8, 3, 4
ER = E * R                       # 32
CH = 256                         # batch chunk per core
NCHUNK = BL // CH                # 8
# chunk plan: 7 full chunks + 2 half chunks (shrinks the exposed tail)
CHUNKS = [(i * CH, CH) for i in range(7)] + [(7 * CH, 128), (7 * CH + 128, 128)]
NIDX = CH * F                    # 8192 gathered rows per chunk
WIDX = NIDX // 16                # 512 idx columns per chunk
NT = 32768                       # compacted table rows (int16-addressable)
EPS, EPS_LN, SCALING = 1e-8, 1e-5, 0.25

_CACHED_NC = None


def _build():
    nc = bacc.Bacc("TRN2", target_bir_lowering=False, debug=False)
    f32, f32r, bf16, i16 = (mybir.dt.float32, mybir.dt.float32r,
                            mybir.dt.bfloat16, mybir.dt.int16)

    tabA = nc.declare_dram_parameter("tabA", [NT, 128], bf16, isOutput=False)
    tabB = nc.declare_dram_parameter("tabB", [NT, 128], bf16, isOutput=False)
    idx_ext = nc.declare_dram_parameter("idx", [128, NCHUNK * WIDX], i16, isOutput=False)
    w0_ext = nc.declare_dram_parameter("w0t", [IN, D0], bf16, isOutput=False)
    w1_ext = nc.declare_dram_parameter("w1t", [D0, D1], bf16, isOutput=False)
    w2_ext = nc.declare_dram_parameter("w2t", [D1, D2], f32r, isOutput=False)
    a0_ext = nc.declare_dram_parameter("a0f", [IN, ER], bf16, isOutput=False)
    a1_ext = nc.declare_dram_parameter("a1f", [D0, ER], bf16, isOutput=False)
    a2_ext = nc.declare_dram_parameter("a2f", [D1, ER], f32r, isOutput=False)
    bm0_ext = nc.declare_dram_parameter("bm0t", [ER, D0], f32r, isOutput=False)
    bm1_ext = nc.declare_dram_parameter("bm1t", [ER, D1], f32r, isOutput=False)
    bm2_ext = nc.declare_dram_parameter("bm2t", [ER, D2], f32r, isOutput=False)
    sc_ext = nc.declare_dram_parameter("scl", [D, 3 * ER], f32r, isOutput=False)
    oh_ext = nc.declare_dram_parameter("onehot", [D, BL], f32r, isOutput=False)
    b0_ext = nc.declare_dram_parameter("b0p", [128, D0 // 128], f32, isOutput=False)
    b1_ext = nc.declare_dram_parameter("b1p", [128, D1 // 128], f32, isOutput=False)
    b2_ext = nc.declare_dram_parameter("b2p", [128, D2 // 128], f32, isOutput=False)
    wt_ext = nc.declare_dram_parameter("wtt", [D2, 64], f32r, isOutput=False)
    bt1_ext = nc.declare_dram_parameter("bt1f", [64, 1], f32, isOutput=False)
    m2_ext = nc.declare_dram_parameter("m2", [64, D], f32r, isOutput=False)
    bt2_ext = nc.declare_dram_parameter("bt2c", [D, 1], f32, isOutput=False)
    ones_ext = nc.declare_dram_parameter("ones8", [D, 1], f32r, isOutput=False)
    out_ext = nc.declare_dram_parameter("out", [1, BL], f32, isOutput=True)

    KT0, KT1, KT2 = IN // 128, D0 // 128, D1 // 128      # 8, 16, 8
    OT0, OT1, OT2 = D0 // 128, D1 // 128, D2 // 128      # 16, 8, 4

    with tile.TileContext(nc) as tc, ExitStack() as ctx:
        wp = ctx.enter_context(tc.tile_pool(name="w", bufs=1))
        gp = ctx.enter_context(tc.tile_pool(name="g", bufs=3))
        xp = ctx.enter_context(tc.tile_pool(name="x", bufs=2))
        hp = ctx.enter_context(tc.tile_pool(name="h", bufs=1))
        sp = ctx.enter_context(tc.tile_pool(name="s", bufs=2))
        pp_tr = ctx.enter_context(tc.tile_pool(name="ptr", bufs=2, space="PSUM"))
        pp_mm = ctx.enter_context(tc.tile_pool(name="pmm", bufs=2, space="PSUM"))
        pp_lo = ctx.enter_context(tc.tile_pool(name="plo", bufs=2, space="PSUM"))
        pp_tw = ctx.enter_context(tc.tile_pool(name="ptw", bufs=2, space="PSUM"))

        ident = wp.tile([128, 128], mybir.dt.bfloat16)
        make_identity(nc, ident[:, :])

        idx_tiles = []
        colw = 0
        for ci, (off, ch) in enumerate(CHUNKS):
            w = (ch * F) // 16
            ixt = wp.tile([128, w], mybir.dt.int16, tag=f"idx{ci}")
            nc.sync.dma_start(out=ixt[:, :], in_=idx_ext[:, colw:colw + w])
            idx_tiles.append(ixt)
            colw += w

        def load_rows(ext, rows, cols, dt, name):
            tiles = []
            for k in range(rows // 128):
                t = wp.tile([128, cols], dt, tag=f"{name}{k}")
                nc.sync.dma_start(out=t[:, :], in_=ext[k * 128:(k + 1) * 128, :])
                tiles.append(t)
            return tiles

        w0t = load_rows(w0_ext, IN, D0, mybir.dt.bfloat16, "w0")
        w1t = load_rows(w1_ext, D0, D1, mybir.dt.bfloat16, "w1")
        w2t = load_rows(w2_ext, D1, D2, mybir.dt.float32r, "w2")
        a0t = load_rows(a0_ext, IN, ER, mybir.dt.bfloat16, "a0")
        a1t = load_rows(a1_ext, D0, ER, mybir.dt.bfloat16, "a1")
        a2t = load_rows(a2_ext, D1, ER, mybir.dt.float32r, "a2")
        wtt = load_rows(wt_ext, D2, 64, mybir.dt.float32r, "wt")

        bm0 = wp.tile([ER, D0], mybir.dt.float32r)
        nc.sync.dma_start(out=bm0[:, :], in_=bm0_ext[:, :])
        bm1 = wp.tile([ER, D1], mybir.dt.float32r)
        nc.sync.dma_start(out=bm1[:, :], in_=bm1_ext[:, :])
        bm2 = wp.tile([ER, D2], mybir.dt.float32r)
        nc.sync.dma_start(out=bm2[:, :], in_=bm2_ext[:, :])
        scl = wp.tile([D, 3 * ER], mybir.dt.float32r)
        nc.sync.dma_start(out=scl[:, :], in_=sc_ext[:, :])
        oh = wp.tile([D, BL], mybir.dt.float32r)
        nc.sync.dma_start(out=oh[:, :], in_=oh_ext[:, :])
        b0p = wp.tile([128, D0 // 128], mybir.dt.float32)
        nc.sync.dma_start(out=b0p[:, :], in_=b0_ext[:, :])
        b1p = wp.tile([128, D1 // 128], mybir.dt.float32)
        nc.sync.dma_start(out=b1p[:, :], in_=b1_ext[:, :])
        b2p = wp.tile([128, D2 // 128], mybir.dt.float32)
        nc.sync.dma_start(out=b2p[:, :], in_=b2_ext[:, :])
        bt1f = wp.tile([64, 1], mybir.dt.float32)
        nc.sync.dma_start(out=bt1f[:, :], in_=bt1_ext[:, :])
        m2 = wp.tile([64, D], mybir.dt.float32r)
        nc.sync.dma_start(out=m2[:, :], in_=m2_ext[:, :])
        bt2c = wp.tile([D, 1], mybir.dt.float32)
        nc.sync.dma_start(out=bt2c[:, :], in_=bt2_ext[:, :])
        ones8 = wp.tile([D, 1], mybir.dt.float32r)
        nc.sync.dma_start(out=ones8[:, :], in_=ones_ext[:, :])

        relu = mybir.ActivationFunctionType.Relu

        def layer(rhs_tile, kt, ot, w_tiles, a_tiles, bm_tile, bias_tile,
                  l_idx, off, ch, out_tile):
            """One FCN layer on a [128, kt*ch] feature-major rhs."""
            # LoRA A-projection: t = A^T h  -> psum [32, ch]
            ps_t = pp_lo.tile([ER, CH], mybir.dt.float32, tag="lo")
            for k in range(kt):
                nc.tensor.matmul(out=ps_t[:, 0:ch], lhsT=a_tiles[k][:, :],
                                 rhs=rhs_tile[:, k * ch:(k + 1) * ch],
                                 start=(k == 0), stop=(k == kt - 1))
            # scale table gathered by domain: s = scl[l].T @ onehot -> [32, ch]
            ps_s = pp_lo.tile([ER, CH], mybir.dt.float32, tag="lo")
            nc.tensor.matmul(out=ps_s[:, 0:ch], lhsT=scl[:, l_idx * ER:(l_idx + 1) * ER],
                             rhs=oh[:, off:off + ch],
                             start=True, stop=True)
            s_sb = sp.tile([ER, CH], mybir.dt.float32r, tag="ssb")
            nc.scalar.activation(out=s_sb[:, 0:ch], in_=ps_s[:, 0:ch],
                                 func=mybir.ActivationFunctionType.Copy)
            t2s = sp.tile([ER, CH], mybir.dt.float32r, tag="t2s")
            nc.vector.tensor_tensor(out=t2s[:, 0:ch], in0=ps_t[:, 0:ch], in1=s_sb[:, 0:ch],
                                    op=mybir.AluOpType.mult)
            for o in range(ot):
                ps = pp_mm.tile([128, CH], mybir.dt.float32, tag="mm")
                for k in range(kt):
                    nc.tensor.matmul(out=ps[:, 0:ch],
                                     lhsT=w_tiles[k][:, o * 128:(o + 1) * 128],
                                     rhs=rhs_tile[:, k * ch:(k + 1) * ch],
                                     start=(k == 0), stop=False)
                nc.tensor.matmul(out=ps[:, 0:ch], lhsT=bm_tile[:, o * 128:(o + 1) * 128],
                                 rhs=t2s[:, 0:ch], start=False, stop=True)
                nc.scalar.activation(out=out_tile[:, o * ch:(o + 1) * ch],
                                     in_=ps[:, 0:ch], func=relu,
                                     bias=bias_tile[:, o:o + 1], scale=1.0)

        def backbone(off, ch, xTc):
            h1c = hp.tile([128, OT0 * CH], mybir.dt.bfloat16, tag="h1")
            layer(xTc, KT0, OT0, w0t, a0t, bm0, b0p, 0, off, ch, h1c)
            h2c = hp.tile([128, OT1 * CH], mybir.dt.float32r, tag="h2")
            layer(h1c, KT1, OT1, w1t, a1t, bm1, b1p, 1, off, ch, h2c)
            h3c = hp.tile([128, OT2 * CH], mybir.dt.float32r, tag="h3")
            layer(h2c, KT2, OT2, w2t, a2t, bm2, b2p, 2, off, ch, h3c)

            # towers: t1 = relu(WtT.T @ h3 + bt1f) [64, ch]
            ps_tw = pp_tw.tile([64, CH], mybir.dt.float32, tag="tw")
            for k in range(OT2):
                nc.tensor.matmul(out=ps_tw[:, 0:ch], lhsT=wtt[k][:, :],
                                 rhs=h3c[:, k * ch:(k + 1) * ch],
                                 start=(k == 0), stop=(k == OT2 - 1))
            t1s = sp.tile([64, CH], mybir.dt.float32r, tag="t1s")
            nc.scalar.activation(out=t1s[:, 0:ch], in_=ps_tw[:, 0:ch], func=relu,
                                 bias=bt1f[:, :], scale=1.0)
            # logits_all = M2.T @ t1 + bt2 -> [8, ch]; mask by onehot; reduce
            ps_l = pp_tw.tile([D, CH], mybir.dt.float32, tag="tw")
            nc.tensor.matmul(out=ps_l[:, 0:ch], lhsT=m2[:, :], rhs=t1s[:, 0:ch],
                             start=True, stop=True)
            lb = sp.tile([D, CH], mybir.dt.float32r, tag="lb")
            nc.vector.tensor_tensor(out=lb[:, 0:ch], in0=ps_l[:, 0:ch],
                                    in1=bt2c[:, :].to_broadcast([D, ch]),
                                    op=mybir.AluOpType.add)
            mk = sp.tile([D, CH], mybir.dt.float32r, tag="mk")
            nc.vector.tensor_tensor(out=mk[:, 0:ch], in0=lb[:, 0:ch],
                                    in1=oh[:, off:off + ch],
                                    op=mybir.AluOpType.mult)
            ps_f = pp_tw.tile([1, CH], mybir.dt.float32, tag="tw")
            nc.tensor.matmul(out=ps_f[:, 0:ch], lhsT=ones8[:, :], rhs=mk[:, 0:ch],
                             start=True, stop=True)
            outc = sp.tile([1, CH], mybir.dt.float32, tag="oc")
            nc.vector.tensor_copy(out=outc[:, 0:ch], in_=ps_f[:, 0:ch])
            nc.sync.dma_start(out=out_ext[0:1, off:off + ch], in_=outc[:, 0:ch])

        # gathers split in two 4096-row halves for finer G-buffer recycling;
        # x_T assembly copies run on ACT so DVE 2-port bursts never lock the
        # GpSimd SWDGE out of its SBUF descriptor rings.
        for ci, (off, ch) in enumerate(CHUNKS):
            tab = tabA if off < BL // 2 else tabB
            ns = ch // 128
            xTc = xp.tile([128, 8 * CH], mybir.dt.bfloat16, tag="xT")
            G = gp.tile([128, NIDX // 128, 128], mybir.dt.bfloat16, tag="G")
            nidx_c = ch * F
            nc.gpsimd.dma_gather(
                out_ap=G[:, 0:nidx_c // 128, :], in_ap=tab[:, :],
                idxs_ap=idx_tiles[ci][:, :],
                num_idxs=nidx_c, num_idxs_reg=nidx_c, elem_size=128,
                transpose=False, single_packet=False)
            for t in range(8):
                ps = pp_tr.tile([128, 256], mybir.dt.bfloat16, tag="tr")
                for s in range(ns):
                    for j in range(4):
                        g = (t * ns + s) * 4 + j
                        nc.tensor.transpose(
                            out=ps[32 * j:32 * (j + 1), 128 * s:128 * (s + 1)],
                            in_=G[:, g, 0:32], identity=ident[:, :],
                            tile_position=(0, 32 * j))
                nc.vector.tensor_copy(out=xTc[:, t * ch:(t + 1) * ch],
                                      in_=ps[:, 0:ch])
            backbone(off, ch, xTc)

    nc.compile()
    return nc


def get_nc():
    global _CACHED_NC
    if _CACHED_NC is None:
        _CACHED_NC = _build()
    return _CACHED_NC


# ---------------- host-side math (exact fp32 mirror of the reference) -------

def _softplus(x):
    return np.logaddexp(0.0, x)


def _ln(x, g, b):
    m = x.mean(-1, keepdims=True)
    v = ((x - m) ** 2).mean(-1, keepdims=True)
    return g * (x - m) / np.sqrt(v + EPS_LN) + b


def _softmax(x):
    e = np.exp(x - x.max(-1, keepdims=True))
    return e / e.sum(-1, keepdims=True)


def _topk_sparse(p, k):
    idx = np.argsort(-p, axis=-1, kind="stable")[..., :k]
    mask = np.zeros_like(p)
    np.put_along_axis(mask, idx, 1.0, axis=-1)
    s = p * mask
    return s / np.maximum(s.sum(-1, keepdims=True), EPS)


def _routing_tables(dom_emb, layer_pos, gate_logits, Wi1, bi1, gi, bti, Wi2,
                    bi2, Wr1, br1, gr, btr, Wr2, br2):
    gate = _softplus(gate_logits.astype(np.float32))
    Rg = np.eye(D, dtype=np.float32) * gate
    Rg = Rg / np.maximum(Rg.sum(1, keepdims=True), EPS)
    hd = Rg @ dom_emb.astype(np.float32)                      # [D, 64]
    ri = np.concatenate([
        np.broadcast_to(hd[:, None, :], (D, L, hd.shape[-1])),
        np.broadcast_to(layer_pos[None].astype(np.float32), (D, L, layer_pos.shape[-1])),
    ], axis=-1)                                               # [D, L, 96]
    hi = np.maximum(_ln(ri @ Wi1.T + bi1, gi, bti), 0.0)
    scores = (hi @ Wi2.T + bi2)[..., 0]
    scores = scores - scores.max(-1, keepdims=True)
    phi = _softmax(scores)
    zeta = _topk_sparse(phi, min(2, L))                       # [D, L]
    hr = np.maximum(_ln(ri @ Wr1.T + br1, gr, btr), 0.0)
    alpha = _topk_sparse(_softmax(hr @ Wr2.T + br2), 2)       # [D, L, E]
    return zeta.astype(np.float32), alpha.astype(np.float32)


def _prep_core(field_idx_c, emb):
    """Compact bf16 gather tables + wrapped int16 device indices for one core."""
    tabs, idx16 = [], []
    half = BL // 2
    for h in range(2):
        fh = field_idx_c[h * half:(h + 1) * half]
        u, inv = np.unique(fh, return_inverse=True)
        tab = np.zeros((NT, 128), dtype=BF16)
        tab[:len(u), 0:ED] = emb[u].astype(BF16)
        tabs.append(tab)
        idx16.append(inv.reshape(fh.shape).astype(np.int16))

    idx_dev = np.zeros((128, NCHUNK * WIDX), dtype=np.int16)
    colw = 0
    for off, ch in CHUNKS:
        ih = idx16[0] if off < half else idx16[1]
        boff = off - (0 if off < half else half)
        blk3 = ih[boff:boff + ch].reshape(ch // 128, 128, 8, 4)   # [s, p, t, j]
        kv = blk3.transpose(2, 0, 3, 1).reshape(ch * F)           # [t, s, j, p]
        w = (ch * F) // 16
        blk = kv.reshape(w, 16).T                                 # [16, w]
        idx_dev[:, colw:colw + w] = np.tile(blk, (8, 1))
        colw += w
    return tabs, idx_dev


def kernel(field_idx, domain_id, emb_table, W0, b0, W1, b1, W2, b2,
           A0, Bm0, A1, Bm1, A2, Bm2, dom_emb, layer_pos, gate_logits,
           Wi1, bi1, gi, bti, Wi2, bi2, Wr1, br1, gr, btr, Wr2, br2,
           Wt1, bt1, Wt2, bt2):
    field_idx = np.asarray(field_idx)
    domain_id = np.asarray(domain_id)
    emb = np.asarray(emb_table, dtype=np.float32)

    zeta, alpha = _routing_tables(
        np.asarray(dom_emb), np.asarray(layer_pos), np.asarray(gate_logits),
        np.asarray(Wi1), np.asarray(bi1), np.asarray(gi), np.asarray(bti),
        np.asarray(Wi2), np.asarray(bi2), np.asarray(Wr1), np.asarray(br1),
        np.asarray(gr), np.asarray(btr), np.asarray(Wr2), np.asarray(br2))

    # per-layer LoRA scale tables packed [D, 3*E*R] (column block per layer)
    scl = np.zeros((D, 3 * ER), dtype=np.float32)
    for l in range(3):
        scl[:, l * ER:(l + 1) * ER] = (
            np.repeat(alpha[:, l, :], R, axis=1) * zeta[:, l, None] * SCALING)

    def prep_w(W):
        return np.ascontiguousarray(np.asarray(W, np.float32).T)

    def prep_a(A, dt):
        return np.ascontiguousarray(
            np.asarray(A, np.float32).transpose(2, 0, 1).reshape(-1, ER)).astype(dt)

    def prep_bm(Bm):
        return np.ascontiguousarray(
            np.asarray(Bm, np.float32).transpose(0, 2, 1).reshape(ER, -1))

    shared = {
        "w0t": prep_w(W0).astype(BF16),
        "w1t": prep_w(W1).astype(BF16),
        "w2t": prep_w(W2),
        "a0f": prep_a(A0, BF16),
        "a1f": prep_a(A1, BF16),
        "a2f": prep_a(A2, np.float32),
        "bm0t": prep_bm(Bm0), "bm1t": prep_bm(Bm1), "bm2t": prep_bm(Bm2),
        "scl": scl,
        "b0p": np.ascontiguousarray(np.asarray(b0, np.float32).reshape(D0 // 128, 128).T),
        "b1p": np.ascontiguousarray(np.asarray(b1, np.float32).reshape(D1 // 128, 128).T),
        "b2p": np.ascontiguousarray(np.asarray(b2, np.float32).reshape(D2 // 128, 128).T),
        "wtt": np.ascontiguousarray(
            np.asarray(Wt1, np.float32).reshape(D * 8, D2).T),
        "bt1f": np.asarray(bt1, np.float32).reshape(64, 1),
        "bt2c": np.asarray(bt2, np.float32).reshape(D, 1),
        "ones8": np.ones((D, 1), np.float32),
    }
    # M2[d*8+o, d'] = Wt2[d, 0, o] iff d == d'
    m2 = np.zeros((64, D), dtype=np.float32)
    wt2 = np.asarray(Wt2, np.float32)
    for d in range(D):
        m2[d * 8:(d + 1) * 8, d] = wt2[d, 0, :]
    shared["m2"] = m2

    in_maps = []
    for ci in range(NCORES):
        sl = slice(ci * BL, (ci + 1) * BL)
        fi = field_idx[sl].astype(np.int64)
        dom = domain_id[sl].astype(np.int64)
        tabs, idx_dev = _prep_core(fi, emb)
        onehot = (dom[None, :] == np.arange(D)[:, None]).astype(np.float32)
        m = dict(shared)
        m.update({"tabA": tabs[0], "tabB": tabs[1], "idx": idx_dev,
                  "onehot": onehot})
        in_maps.append(m)

    nc = get_nc()
    res = bass_utils.run_bass_kernel_spmd(nc, in_maps, core_ids=list(range(NCORES)))
    out = np.concatenate([np.asarray(res.results[i]["out"][0], np.float32)
                          for i in range(NCORES)])
    return out


# revision 12
# speedup vs baseline: 1.0266x; 1.0266x over previous
"""Trainium2 Bass kernel for nn_ADLS_13022340842024 (moe_routing).

Data-parallel over batch across 8 NeuronCores (2048 samples/core).

Key algorithmic reductions (host-side, weight/index-only prep):
  * The gated domain-relation matrix Rg is a row-normalized diagonal =>
    h_prime = dom_emb[domain_id] exactly; all hierarchical routing (zeta,
    alpha) is therefore a function of domain_id only -> tiny [D,L]/[D,L,E]
    tables folded with SCALING into per-layer LoRA scale tables [D, E*R].
  * Per-domain towers flattened to one [512,64] matmul + block-diagonal
    [64,8] second layer + one-hot select.

On-device per core:
  * Embedding gather via dma_gather from per-half-batch compacted bf16
    tables (256B rows), PE col-packed transposes -> x_T [feat, batch].
  * 3-layer FCN backbone as feature-major matmuls (bf16 L0/L1, float32r
    L2) with LoRA experts folded into the PSUM accumulation, ACT
    relu+bias epilogues.
  * Towers + one-hot domain select, PE ones-reduce.
"""
import numpy as np
import ml_dtypes
from contextlib import ExitStack

import concourse.bass as bass
import concourse.tile as tile
from concourse import bacc, mybir
from concourse import bass_utils
from concourse.masks import make_identity

BF16 = ml_dtypes.bfloat16

B, F, V, ED = 16384, 32, 100000, 32
NCORES = 8
BL = B // NCORES                 # 2048 samples per core
IN, D0, D1, D2 = 1024, 2048, 1024, 512
D, E, L, R = 8, 8, 3, 4
ER = E * R                       # 32
CH = 256                         # batch chunk per core
NCHUNK = BL // CH                # 8
NIDX = CH * F                    # 8192 gathered rows per chunk
WIDX = NIDX // 16                # 512 idx columns per chunk
NT = 32768                       # compacted table rows (int16-addressable)
EPS, EPS_LN, SCALING = 1e-8, 1e-5, 0.25

_CACHED_NC = None


def _build():
    nc = bacc.Bacc("TRN2", target_bir_lowering=False, debug=False)
    f32, f32r, bf16, i16 = (mybir.dt.float32, mybir.dt.float32r,
                            mybir.dt.bfloat16, mybir.dt.int16)

    tabA = nc.declare_dram_parameter("tabA", [NT, 128], bf16, isOutput=False)
    tabB = nc.declare_dram_parameter("tabB", [NT, 128], bf16, isOutput=False)
    idx_ext = nc.declare_dram_parameter("idx", [128, NCHUNK * WIDX], i16, isOutput=False)
    w0_ext = nc.declare_dram_parameter("w0t", [IN, D0], bf16, isOutput=False)
    w1_ext = nc.declare_dram_parameter("w1t", [D0, D1], bf16, isOutput=False)
    w2_ext = nc.declare_dram_parameter("w2t", [D1, D2], f32r, isOutput=False)
    a0_ext = nc.declare_dram_parameter("a0f", [IN, ER], bf16, isOutput=False)
    a1_ext = nc.declare_dram_parameter("a1f", [D0, ER], bf16, isOutput=False)
    a2_ext = nc.declare_dram_parameter("a2f", [D1, ER], f32r, isOutput=False)
    bm0_ext = nc.declare_dram_parameter("bm0t", [ER, D0], f32r, isOutput=False)
    bm1_ext = nc.declare_dram_parameter("bm1t", [ER, D1], f32r, isOutput=False)
    bm2_ext = nc.declare_dram_parameter("bm2t", [ER, D2], f32r, isOutput=False)
    sc_ext = nc.declare_dram_parameter("scl", [D, 3 * ER], f32r, isOutput=False)
    oh_ext = nc.declare_dram_parameter("onehot", [D, BL], f32r, isOutput=False)
    b0_ext = nc.declare_dram_parameter("b0p", [128, D0 // 128], f32, isOutput=False)
    b1_ext = nc.declare_dram_parameter("b1p", [128, D1 // 128], f32, isOutput=False)
    b2_ext = nc.declare_dram_parameter("b2p", [128, D2 // 128], f32, isOutput=False)
    wt_ext = nc.declare_dram_parameter("wtt", [D2, 64], f32r, isOutput=False)
    bt1_ext = nc.declare_dram_parameter("bt1f", [64, 1], f32, isOutput=False)
    m2_ext = nc.declare_dram_parameter("m2", [64, D], f32r, isOutput=False)
    bt2_ext = nc.declare_dram_parameter("bt2c", [D, 1], f32, isOutput=False)
    ones_ext = nc.declare_dram_parameter("ones8", [D, 1], f32r, isOutput=False)
    out_ext = nc.declare_dram_parameter("out", [1, BL], f32, isOutput=True)

    KT0, KT1, KT2 = IN // 128, D0 // 128, D1 // 128      # 8, 16, 8
    OT0, OT1, OT2 = D0 // 128, D1 // 128, D2 // 128      # 16, 8, 4

    with tile.TileContext(nc) as tc, ExitStack() as ctx:
        wp = ctx.enter_context(tc.tile_pool(name="w", bufs=1))
        gp = ctx.enter_context(tc.tile_pool(name="g", bufs=3))
        xp = ctx.enter_context(tc.tile_pool(name="x", bufs=2))
        hp = ctx.enter_context(tc.tile_pool(name="h", bufs=1))
        sp = ctx.enter_context(tc.tile_pool(name="s", bufs=2))
        pp_tr = ctx.enter_context(tc.tile_pool(name="ptr", bufs=2, space="PSUM"))
        pp_mm = ctx.enter_context(tc.tile_pool(name="pmm", bufs=2, space="PSUM"))
        pp_lo = ctx.enter_context(tc.tile_pool(name="plo", bufs=2, space="PSUM"))
        pp_tw = ctx.enter_context(tc.tile_pool(name="ptw", bufs=2, space="PSUM"))

        ident = wp.tile([128, 128], mybir.dt.bfloat16)
        make_identity(nc, ident[:, :])

        idx_tiles = []
        for c in range(NCHUNK):
            ixt = wp.tile([128, WIDX], mybir.dt.int16, tag=f"idx{c}")
            nc.sync.dma_start(out=ixt[:, :], in_=idx_ext[:, c * WIDX:(c + 1) * WIDX])
            idx_tiles.append(ixt)

        def load_rows(ext, rows, cols, dt, name):
            tiles = []
            for k in range(rows // 128):
                t = wp.tile([128, cols], dt, tag=f"{name}{k}")
                nc.sync.dma_start(out=t[:, :], in_=ext[k * 128:(k + 1) * 128, :])
                tiles.append(t)
            return tiles

        w0t = load_rows(w0_ext, IN, D0, mybir.dt.bfloat16, "w0")
        w1t = load_rows(w1_ext, D0, D1, mybir.dt.bfloat16, "w1")
        w2t = load_rows(w2_ext, D1, D2, mybir.dt.float32r, "w2")
        a0t = load_rows(a0_ext, IN, ER, mybir.dt.bfloat16, "a0")
        a1t = load_rows(a1_ext, D0, ER, mybir.dt.bfloat16, "a1")
        a2t = load_rows(a2_ext, D1, ER, mybir.dt.float32r, "a2")
        wtt = load_rows(wt_ext, D2, 64, mybir.dt.float32r, "wt")

        bm0 = wp.tile([ER, D0], mybir.dt.float32r)
        nc.sync.dma_start(out=bm0[:, :], in_=bm0_ext[:, :])
        bm1 = wp.tile([ER, D1], mybir.dt.float32r)
        nc.sync.dma_start(out=bm1[:, :], in_=bm1_ext[:, :])
        bm2 = wp.tile([ER, D2], mybir.dt.float32r)
        nc.sync.dma_start(out=bm2[:, :], in_=bm2_ext[:, :])
        scl = wp.tile([D, 3 * ER], mybir.dt.float32r)
        nc.sync.dma_start(out=scl[:, :], in_=sc_ext[:, :])
        oh = wp.tile([D, BL], mybir.dt.float32r)
        nc.sync.dma_start(out=oh[:, :], in_=oh_ext[:, :])
        b0p = wp.tile([128, D0 // 128], mybir.dt.float32)
        nc.sync.dma_start(out=b0p[:, :], in_=b0_ext[:, :])
        b1p = wp.tile([128, D1 // 128], mybir.dt.float32)
        nc.sync.dma_start(out=b1p[:, :], in_=b1_ext[:, :])
        b2p = wp.tile([128, D2 // 128], mybir.dt.float32)
        nc.sync.dma_start(out=b2p[:, :], in_=b2_ext[:, :])
        bt1f = wp.tile([64, 1], mybir.dt.float32)
        nc.sync.dma_start(out=bt1f[:, :], in_=bt1_ext[:, :])
        m2 = wp.tile([64, D], mybir.dt.float32r)
        nc.sync.dma_start(out=m2[:, :], in_=m2_ext[:, :])
        bt2c = wp.tile([D, 1], mybir.dt.float32)
        nc.sync.dma_start(out=bt2c[:, :], in_=bt2_ext[:, :])
        ones8 = wp.tile([D, 1], mybir.dt.float32r)
        nc.sync.dma_start(out=ones8[:, :], in_=ones_ext[:, :])

        relu = mybir.ActivationFunctionType.Relu

        def layer(rhs_tile, kt, ot, w_tiles, a_tiles, bm_tile, bias_tile,
                  l_idx, chunk, out_tile, out_dt_bits):
            """One FCN layer on a [128, kt*CH] feature-major rhs."""
            # LoRA A-projection: t = A^T h  -> psum [32, CH]
            ps_t = pp_lo.tile([ER, CH], mybir.dt.float32, tag="lo")
            for k in range(kt):
                nc.tensor.matmul(out=ps_t[:, :], lhsT=a_tiles[k][:, :],
                                 rhs=rhs_tile[:, k * CH:(k + 1) * CH],
                                 start=(k == 0), stop=(k == kt - 1))
            # scale table gathered by domain: s = scl[l].T @ onehot -> [32, CH]
            ps_s = pp_lo.tile([ER, CH], mybir.dt.float32, tag="lo")
            nc.tensor.matmul(out=ps_s[:, :], lhsT=scl[:, l_idx * ER:(l_idx + 1) * ER],
                             rhs=oh[:, chunk * CH:(chunk + 1) * CH],
                             start=True, stop=True)
            s_sb = sp.tile([ER, CH], mybir.dt.float32r, tag="ssb")
            nc.scalar.activation(out=s_sb[:, :], in_=ps_s[:, :],
                                 func=mybir.ActivationFunctionType.Copy)
            t2s = sp.tile([ER, CH], mybir.dt.float32r, tag="t2s")
            nc.vector.tensor_tensor(out=t2s[:, :], in0=ps_t[:, :], in1=s_sb[:, :],
                                    op=mybir.AluOpType.mult)
            for o in range(ot):
                ps = pp_mm.tile([128, CH], mybir.dt.float32, tag="mm")
                for k in range(kt):
                    nc.tensor.matmul(out=ps[:, :],
                                     lhsT=w_tiles[k][:, o * 128:(o + 1) * 128],
                                     rhs=rhs_tile[:, k * CH:(k + 1) * CH],
                                     start=(k == 0), stop=False)
                nc.tensor.matmul(out=ps[:, :], lhsT=bm_tile[:, o * 128:(o + 1) * 128],
                                 rhs=t2s[:, :], start=False, stop=True)
                nc.scalar.activation(out=out_tile[:, o * CH:(o + 1) * CH],
                                     in_=ps[:, :], func=relu,
                                     bias=bias_tile[:, o:o + 1], scale=1.0)

        def backbone(c, xTc):
            h1c = hp.tile([128, OT0 * CH], mybir.dt.bfloat16, tag="h1")
            layer(xTc, KT0, OT0, w0t, a0t, bm0, b0p, 0, c, h1c, 16)
            h2c = hp.tile([128, OT1 * CH], mybir.dt.float32r, tag="h2")
            layer(h1c, KT1, OT1, w1t, a1t, bm1, b1p, 1, c, h2c, 32)
            h3c = hp.tile([128, OT2 * CH], mybir.dt.float32r, tag="h3")
            layer(h2c, KT2, OT2, w2t, a2t, bm2, b2p, 2, c, h3c, 32)

            # towers: t1 = relu(WtT.T @ h3 + bt1f) [64, CH]
            ps_tw = pp_tw.tile([64, CH], mybir.dt.float32, tag="tw")
            for k in range(OT2):
                nc.tensor.matmul(out=ps_tw[:, :], lhsT=wtt[k][:, :],
                                 rhs=h3c[:, k * CH:(k + 1) * CH],
                                 start=(k == 0), stop=(k == OT2 - 1))
            t1s = sp.tile([64, CH], mybir.dt.float32r, tag="t1s")
            nc.scalar.activation(out=t1s[:, :], in_=ps_tw[:, :], func=relu,
                                 bias=bt1f[:, :], scale=1.0)
            # logits_all = M2.T @ t1 + bt2 -> [8, CH]; mask by onehot; reduce
            ps_l = pp_tw.tile([D, CH], mybir.dt.float32, tag="tw")
            nc.tensor.matmul(out=ps_l[:, :], lhsT=m2[:, :], rhs=t1s[:, :],
                             start=True, stop=True)
            lb = sp.tile([D, CH], mybir.dt.float32r, tag="lb")
            nc.vector.tensor_tensor(out=lb[:, :], in0=ps_l[:, :],
                                    in1=bt2c[:, :].to_broadcast([D, CH]),
                                    op=mybir.AluOpType.add)
            mk = sp.tile([D, CH], mybir.dt.float32r, tag="mk")
            nc.vector.tensor_tensor(out=mk[:, :], in0=lb[:, :],
                                    in1=oh[:, c * CH:(c + 1) * CH],
                                    op=mybir.AluOpType.mult)
            ps_f = pp_tw.tile([1, CH], mybir.dt.float32, tag="tw")
            nc.tensor.matmul(out=ps_f[:, :], lhsT=ones8[:, :], rhs=mk[:, :],
                             start=True, stop=True)
            outc = sp.tile([1, CH], mybir.dt.float32, tag="oc")
            nc.vector.tensor_copy(out=outc[:, :], in_=ps_f[:, :])
            nc.sync.dma_start(out=out_ext[0:1, c * CH:(c + 1) * CH], in_=outc[:, :])

        # gathers split in two 4096-row halves for finer G-buffer recycling;
        # x_T assembly copies run on ACT so DVE 2-port bursts never lock the
        # GpSimd SWDGE out of its SBUF descriptor rings.
        for c in range(NCHUNK):
            tab = tabA if c < NCHUNK // 2 else tabB
            xTc = xp.tile([128, 8 * CH], mybir.dt.bfloat16, tag="xT")
            G = gp.tile([128, NIDX // 128, 128], mybir.dt.bfloat16, tag="G")
            nc.gpsimd.dma_gather(
                out_ap=G[:, :, :], in_ap=tab[:, :],
                idxs_ap=idx_tiles[c][:, :],
                num_idxs=NIDX, num_idxs_reg=NIDX, elem_size=128,
                transpose=False, single_packet=False)
            for t in range(8):
                ps = pp_tr.tile([128, 256], mybir.dt.bfloat16, tag="tr")
                for s in range(2):
                    for j in range(4):
                        g = (t * 2 + s) * 4 + j
                        nc.tensor.transpose(
                            out=ps[32 * j:32 * (j + 1), 128 * s:128 * (s + 1)],
                            in_=G[:, g, 0:32], identity=ident[:, :],
                            tile_position=(0, 32 * j))
                nc.vector.tensor_copy(out=xTc[:, t * 256:(t + 1) * 256],
                                      in_=ps[:, :])
            backbone(c, xTc)

    nc.compile()
    return nc


def get_nc():
    global _CACHED_NC
    if _CACHED_NC is None:
        _CACHED_NC = _build()
    return _CACHED_NC


# ---------------- host-side math (exact fp32 mirror of the reference) -------

def _softplus(x):
    return np.logaddexp(0.0, x)


def _ln(x, g, b):
    m = x.mean(-1, keepdims=True)
    v = ((x - m) ** 2).mean(-1, keepdims=True)
    return g * (x - m) / np.sqrt(v + EPS_LN) + b


def _softmax(x):
    e = np.exp(x - x.max(-1, keepdims=True))
    return e / e.sum(-1, keepdims=True)


def _topk_sparse(p, k):
    idx = np.argsort(-p, axis=-1, kind="stable")[..., :k]
    mask = np.zeros_like(p)
    np.put_along_axis(mask, idx, 1.0, axis=-1)
    s = p * mask
    return s / np.maximum(s.sum(-1, keepdims=True), EPS)


def _routing_tables(dom_emb, layer_pos, gate_logits, Wi1, bi1, gi, bti, Wi2,
                    bi2, Wr1, br1, gr, btr, Wr2, br2):
    gate = _softplus(gate_logits.astype(np.float32))
    Rg = np.eye(D, dtype=np.float32) * gate
    Rg = Rg / np.maximum(Rg.sum(1, keepdims=True), EPS)
    hd = Rg @ dom_emb.astype(np.float32)                      # [D, 64]
    ri = np.concatenate([
        np.broadcast_to(hd[:, None, :], (D, L, hd.shape[-1])),
        np.broadcast_to(layer_pos[None].astype(np.float32), (D, L, layer_pos.shape[-1])),
    ], axis=-1)                                               # [D, L, 96]
    hi = np.maximum(_ln(ri @ Wi1.T + bi1, gi, bti), 0.0)
    scores = (hi @ Wi2.T + bi2)[..., 0]
    scores = scores - scores.max(-1, keepdims=True)
    phi = _softmax(scores)
    zeta = _topk_sparse(phi, min(2, L))                       # [D, L]
    hr = np.maximum(_ln(ri @ Wr1.T + br1, gr, btr), 0.0)
    alpha = _topk_sparse(_softmax(hr @ Wr2.T + br2), 2)       # [D, L, E]
    return zeta.astype(np.float32), alpha.astype(np.float32)


def _prep_core(field_idx_c, emb):
    """Compact bf16 gather tables + wrapped int16 device indices for one core."""
    tabs, idx16 = [], []
    half = BL // 2
    for h in range(2):
        fh = field_idx_c[h * half:(h + 1) * half]
        u, inv = np.unique(fh, return_inverse=True)
        tab = np.zeros((NT, 128), dtype=BF16)
        tab[:len(u), 0:ED] = emb[u].astype(BF16)
        tabs.append(tab)
        idx16.append(inv.reshape(fh.shape).astype(np.int16))

    idx_dev = np.zeros((128, NCHUNK * WIDX), dtype=np.int16)
    ch_per_half = NCHUNK // 2
    for c in range(NCHUNK):
        ih = idx16[c // ch_per_half]
        boff = (c % ch_per_half) * CH
        blk3 = ih[boff:boff + CH].reshape(2, 128, 8, 4)       # [s, p, t, j]
        kv = blk3.transpose(2, 0, 3, 1).reshape(NIDX)         # [t, s, j, p]
        blk = kv.reshape(WIDX, 16).T                          # [16, WIDX]
        idx_dev[:, c * WIDX:(c + 1) * WIDX] = np.tile(blk, (8, 1))
    return tabs, idx_dev


def kernel(field_idx, domain_id, emb_table, W0, b0, W1, b1, W2, b2,
           A0, Bm0, A1, Bm1, A2, Bm2, dom_emb, layer_pos, gate_logits,
           Wi1, bi1, gi, bti, Wi2, bi2, Wr1, br1, gr, btr, Wr2, br2,
           Wt1, bt1, Wt2, bt2):
    field_idx = np.asarray(field_idx)
    domain_id = np.asarray(domain_id)
    emb = np.asarray(emb_table, dtype=np.float32)

    zeta, alpha = _routing_tables(
        np.asarray(dom_emb), np.asarray(layer_pos), np.asarray(gate_logits),
        np.asarray(Wi1), np.asarray(bi1), np.asarray(gi), np.asarray(bti),
        np.asarray(Wi2), np.asarray(bi2), np.asarray(Wr1), np.asarray(br1),
        np.asarray(gr), np.asarray(btr), np.asarray(Wr2), np.asarray(br2))

    # per-layer LoRA scale tables packed [D, 3*E*R] (column block per layer)
    scl = np.zeros((D, 3 * ER), dtype=np.float32)
    for l in range(3):
        scl[:, l * ER:(l + 1) * ER] = (
            np.repeat(alpha[:, l, :], R, axis=1) * zeta[:, l, None] * SCALING)

    def prep_w(W):
        return np.ascontiguousarray(np.asarray(W, np.float32).T)

    def prep_a(A, dt):
        return np.ascontiguousarray(
            np.asarray(A, np.float32).transpose(2, 0, 1).reshape(-1, ER)).astype(dt)

    def prep_bm(Bm):
        return np.ascontiguousarray(
            np.asarray(Bm, np.float32).transpose(0, 2, 1).reshape(ER, -1))

    shared = {
        "w0t": prep_w(W0).astype(BF16),
        "w1t": prep_w(W1).astype(BF16),
        "w2t": prep_w(W2),
        "a0f": prep_a(A0, BF16),
        "a1f": prep_a(A1, BF16),
        "a2f": prep_a(A2, np.float32),
        "bm0t": prep_bm(Bm0), "bm1t": prep_bm(Bm1), "bm2t": prep_bm(Bm2),
        "scl": scl,
        "b0p": np.ascontiguousarray(np.asarray(b0, np.float32).reshape(D0 // 128, 128).T),
        "b1p": np.ascontiguousarray(np.asarray(b1, np.float32).reshape(D1 // 128, 128).T),
        "b2p": np.ascontiguousarray(np.asarray(b2, np.float32).reshape(D2 // 128, 128).T),
        "wtt": np.ascontiguousarray(
            np.asarray(Wt1, np.float32).reshape(D * 8, D2).T),
        "bt1f": np.asarray(bt1, np.float32).reshape(64, 1),
        "bt2c": np.asarray(bt2, np.float32).reshape(D, 1),
        "ones8": np.ones((D, 1), np.float32),
    }
    # M2[d*8+o, d'] = Wt2[d, 0, o] iff d == d'
    m2 = np.zeros((64, D), dtype=np.float32)
    wt2 = np.asarray(Wt2, np.float32)
    for d in range(D):
        m2[d * 8:(d + 1) * 8, d] = wt2[d, 0, :]
    shared["m2"] = m2

    in_maps = []
    for ci in range(NCORES):
        sl = slice(ci * BL, (ci + 1) * BL)
        fi = field_idx[sl].astype(np.int64)
        dom = domain_id[sl].astype(np.int64)
        tabs, idx_dev = _prep_core(fi, emb)
        onehot = (dom[None, :] == np.arange(D)[:, None]).astype(np.float32)
        m = dict(shared)
        m.update({"tabA": tabs[0], "tabB": tabs[1], "idx": idx_dev,
                  "onehot": onehot})
        in_maps.append(m)

    nc = get_nc()
    res = bass_utils.run_bass_kernel_spmd(nc, in_maps, core_ids=list(range(NCORES)))
    out = np.concatenate([np.asarray(res.results[i]["out"][0], np.float32)
                          for i in range(NCORES)])
    return out
